# revision 1
# baseline (speedup 1.0000x reference)
"""GQA attention (B=2, S=2048, D=2048, H=32, G=8, hd=64) on 8 TRN2 cores.

ZERO-COLLECTIVE sharding: core c owns (batch b=c//4, token block q0=512*(c%4)).
Each core computes the FULL output slice out[b, q0:q0+512, :] independently:
full K/V over all S (kv projection replicated within a batch group), Q only
for its own 512 tokens, attention + output projection fully local. No
cross-core communication, no barriers -> per-core span is immune to
cross-core start skew.

Per-core SPMD uniformity: the token axis of x^T (and the RoPE tables) is
ROTATED by q0 on the host, so "own tokens" are always columns 0:512.
Attention is permutation-invariant over keys; RoPE phases ride with the
rotation.

Layouts (host-side transposes):
  - x^T resident in SBUF [128, 16, S]; K^T [128(2 groups), 4, S] and
    V [tok-part, feat] projected on-chip; V is stored augmented with a ones
    column per head so the PV matmul accumulates the softmax denominator in
    PSUM row 64 for free. 1/d via exp(-ln(d)), folded into o^T before the
    output projection.
  - scores come out transposed [k, q] so PV needs no on-chip transposes.
  - head pair (hA=8t+r, hB=8t+4+r) processed together: one [128,1024] PSUM
    scores tile (A|B), one exp() activation covers both heads.
"""

import sys

sys.path.insert(0, "/opt/trn_rl_repo")

import numpy as np
import ml_dtypes

import concourse.bass as bass
import concourse.tile as tile
from concourse import bacc, mybir
from concourse.bass_utils import run_bass_kernel_spmd

BF16 = ml_dtypes.bfloat16
B, S, D = 2, 2048, 2048
H, G, HD = 32, 8, 64
DC = D // 128  # 16 dim chunks
N_CORES = 8
TOK = 512  # own tokens per core

_CACHE = {}


def _build():
    f32 = mybir.dt.float32
    bf16 = mybir.dt.bfloat16
    nc = bacc.Bacc("TRN2", target_bir_lowering=False, debug=False, num_devices=N_CORES)

    xt = nc.dram_tensor("xt", [128, DC, S], bf16, kind="ExternalInput").ap()
    wq = nc.dram_tensor("wq", [128, DC, DC, 128], bf16, kind="ExternalInput").ap()
    wk = nc.dram_tensor("wk", [128, DC, 4, 128], bf16, kind="ExternalInput").ap()
    wv = nc.dram_tensor("wv", [128, DC, 512], bf16, kind="ExternalInput").ap()
    cosr = nc.dram_tensor("cosr", [128, S], bf16, kind="ExternalInput").ap()
    sinr = nc.dram_tensor("sinr", [128, S], bf16, kind="ExternalInput").ap()
    wo = nc.dram_tensor("wo", [128, DC, D], bf16, kind="ExternalInput").ap()
    out = nc.dram_tensor("out", [TOK, D], f32, kind="ExternalOutput").ap()

    Exp = mybir.ActivationFunctionType.Exp
    Ln = mybir.ActivationFunctionType.Ln
    swap_mask = [i ^ 1 for i in range(32)]
    scale = float(1.0 / np.sqrt(HD))

    from contextlib import ExitStack
    with tile.TileContext(nc) as tc, ExitStack() as ctx:
        consts = ctx.enter_context(tc.tile_pool(name="consts", bufs=1))
        wqp = ctx.enter_context(tc.tile_pool(name="wqp", bufs=2))
        io = ctx.enter_context(tc.tile_pool(name="io", bufs=2))
        work = ctx.enter_context(tc.tile_pool(name="work", bufs=3))
        outw = ctx.enter_context(tc.tile_pool(name="outw", bufs=2))
        # scores get their own 2x[128,1024] pool (4 banks); projection
        # accumulators + PV accumulators + outproj share a 4x[*,512] pool
        # (4 banks) so attention overlaps the projections.
        psum = ctx.enter_context(tc.tile_pool(name="psum", bufs=2, space="PSUM"))
        opsum = ctx.enter_context(tc.tile_pool(name="opsum", bufs=4, space="PSUM"))
        dram = ctx.enter_context(tc.tile_pool(name="dram", bufs=1, space="DRAM"))

        # ---- load inputs, ordered for the earliest possible first vproj
        # matmul: wv + x^T chunks feed it; wk/cos/sin follow
        wv_sb = consts.tile([128, DC, 512], bf16, tag="wv")
        nc.sync.dma_start(out=wv_sb[:], in_=wv[:])
        xt_sb = consts.tile([128, DC, S], bf16, tag="xt")
        nc.sync.dma_start(out=xt_sb[:, 0:2, :], in_=xt[:, 0:2, :])
        wk_sb = consts.tile([128, DC, 4, 128], bf16, tag="wk")
        nc.sync.dma_start(out=wk_sb[:], in_=wk[:])
        for i in range(1, 8):
            nc.sync.dma_start(out=xt_sb[:, 2 * i:2 * i + 2, :],
                              in_=xt[:, 2 * i:2 * i + 2, :])
        cos_sb = consts.tile([128, S], bf16, tag="cos")
        nc.sync.dma_start(out=cos_sb[:], in_=cosr[:])
        sin_sb = consts.tile([128, S], bf16, tag="sin")
        nc.sync.dma_start(out=sin_sb[:], in_=sinr[:])

        kt_sb = consts.tile([128, 4, S], bf16, tag="kt")
        vaug_sb = consts.tile([128, DC, 520], bf16, tag="vaug")
        qt_sb = consts.tile([128, DC, TOK], bf16, tag="qt")
        # denominator staging: pair fc -> partition 32*(fc%4), free block fc//4
        # (DVE partition bases must be 32-aligned). memset(1) keeps the unused
        # rows finite through the Ln/Exp pass.
        dstage = consts.tile([97, 4096], f32, tag="dstage")
        nc.vector.memset(dstage[:], 1.0)
        ddram = dram.tile([4, 4096], f32, tag="dd", name="dd")

        # ones columns of augmented V (130t+64 for head A, 130t+129 for B)
        for t in range(4):
            nc.vector.memset(vaug_sb[:, :, 130 * t + 64:130 * t + 65], 1.0)
            nc.vector.memset(vaug_sb[:, :, 130 * t + 129:130 * t + 130], 1.0)

        def rope(ap, cs, sn):
            sw = io.tile([128, 512], bf16, tag="rsw")
            nc.vector.stream_shuffle(sw, ap, swap_mask)
            nc.vector.tensor_mul(sw, sw, sn)
            tmp = io.tile([128, 512], bf16, tag="rtmp")
            nc.vector.tensor_mul(tmp, ap, cs)
            nc.vector.tensor_add(ap, sw, tmp)

        # ---- V projection into augmented layout (first: attention streams it)
        for tb in range(DC):
            ps = opsum.tile([128, 512], f32, tag="o")
            for c in range(DC):
                nc.tensor.matmul(
                    ps,
                    lhsT=xt_sb[:, c, tb * 128:(tb + 1) * 128],
                    rhs=wv_sb[:, c, :],
                    start=(c == 0), stop=(c == DC - 1),
                )
            for t in range(4):
                nc.vector.tensor_copy(
                    vaug_sb[:, tb, 130 * t:130 * t + 64],
                    ps[:, t * 128:t * 128 + 64])
                nc.vector.tensor_copy(
                    vaug_sb[:, tb, 130 * t + 65:130 * t + 129],
                    ps[:, t * 128 + 64:t * 128 + 128])

        # ot reuses the wv slot (wv dead after vproj)
        ot_sb = consts.tile([128, DC, TOK], bf16, tag="wv")

        def recip_quarter(t):
            # 1/d for pairs fc in [4t, 4t+4): dstage free cols [1024t, 1024t+1024)
            hs = slice(1024 * t, 1024 * (t + 1))
            nc.scalar.activation(dstage[:, hs], dstage[:, hs], Ln)
            nc.scalar.activation(dstage[:, hs], dstage[:, hs], Exp, scale=-1.0)
            for rr in range(4):
                nc.sync.dma_start(out=ddram[rr:rr + 1, hs],
                                  in_=dstage[32 * rr:32 * rr + 1, hs])
            for fc in range(4 * t, 4 * t + 4):
                dp = fc % 4
                df = (fc // 4) * 1024
                r2 = io.tile([128, TOK], bf16, tag="r2")
                nc.gpsimd.dma_start(
                    out=r2[0:64, :],
                    in_=ddram[dp:dp + 1, df:df + 512].partition_broadcast(64))
                nc.gpsimd.dma_start(
                    out=r2[64:128, :],
                    in_=ddram[dp:dp + 1, df + 512:df + 1024].partition_broadcast(64))
                nc.vector.tensor_mul(ot_sb[:, fc, :], ot_sb[:, fc, :], r2)

        # ---- rounds: projections emitted ONE ROUND AHEAD of the attention
        # that consumes them, so TensorE's slack during ACT-bound attention
        # prefetches the next round and exp() never waits at round boundaries.
        def proj_round(t):
            for ssl in range(4):
                # one K chain, then one Q chain: the K-RoPE DVE latency
                # hides under the next chain's matmuls
                sl = slice(ssl * 512, (ssl + 1) * 512)
                ps = opsum.tile([128, 512], f32, tag="o")
                for c in range(DC):
                    nc.tensor.matmul(
                        ps,
                        lhsT=wk_sb[:, c, t, :],
                        rhs=xt_sb[:, c, sl],
                        start=(c == 0), stop=(c == DC - 1),
                    )
                nc.vector.tensor_copy(kt_sb[:, t, sl], ps)
                rope(kt_sb[:, t, sl], cos_sb[:, sl], sin_sb[:, sl])
                fc = 4 * t + ssl
                wq_t = wqp.tile([128, DC, 128], bf16, tag="wq")
                nc.sync.dma_start(out=wq_t[:], in_=wq[:, fc, :, :])
                ps = opsum.tile([128, 512], f32, tag="o")
                for c in range(DC):
                    nc.tensor.matmul(
                        ps,
                        lhsT=wq_t[:, c, :],
                        rhs=xt_sb[:, c, 0:TOK],
                        start=(c == 0), stop=(c == DC - 1),
                    )
                nc.vector.tensor_copy(qt_sb[:, fc, :], ps)
                rope(qt_sb[:, fc, :], cos_sb[:, 0:TOK], sin_sb[:, 0:TOK])

        def attn_round(t, r0, r1):
            # attention pairs of this t: heads (8t+r | 8t+4+r), fc = 4t+r
            for r in range(r0, r1):
                fc = 4 * t + r
                oA = opsum.tile([65, 512], f32, tag="o")
                oB = opsum.tile([65, 512], f32, tag="o")
                for kb in range(DC):
                    ksl = slice(kb * 128, (kb + 1) * 128)
                    s = psum.tile([128, 1024], f32, tag="s")
                    nc.tensor.matmul(
                        s[:, 0:512], lhsT=kt_sb[0:64, t, ksl],
                        rhs=qt_sb[0:64, fc, :],
                        start=True, stop=True, tile_position=(0, 0),
                    )
                    nc.tensor.matmul(
                        s[:, 512:1024], lhsT=kt_sb[64:128, t, ksl],
                        rhs=qt_sb[64:128, fc, :],
                        start=True, stop=True, tile_position=(64, 0),
                    )
                    p = work.tile([128, 1024], bf16, tag="p")
                    nc.scalar.activation(p, s, Exp, scale=scale)
                    nc.tensor.matmul(
                        oA, lhsT=vaug_sb[:, kb, 130 * t:130 * t + 65],
                        rhs=p[:, 0:512],
                        start=(kb == 0), stop=(kb == DC - 1),
                    )
                    nc.tensor.matmul(
                        oB, lhsT=vaug_sb[:, kb, 130 * t + 65:130 * t + 130],
                        rhs=p[:, 512:1024],
                        start=(kb == 0), stop=(kb == DC - 1),
                    )
                nc.vector.tensor_copy(ot_sb[0:64, fc, :], oA[0:64, :])
                nc.vector.tensor_copy(ot_sb[64:128, fc, :], oB[0:64, :])
                dp = 32 * (fc % 4)
                df = (fc // 4) * 1024
                nc.vector.tensor_copy(
                    dstage[dp:dp + 1, df:df + 512], oA[64:65, :])
                nc.vector.tensor_copy(
                    dstage[dp:dp + 1, df + 512:df + 1024], oB[64:65, :])


        proj_round(0)
        attn_round(0, 0, 2)
        proj_round(1)
        attn_round(0, 2, 4)
        recip_quarter(0)
        proj_round(2)
        attn_round(1, 0, 4)
        recip_quarter(1)
        proj_round(3)
        # wo reuses the xt slot (xt dead after the last projections)
        wo_sb = consts.tile([128, DC, D], bf16, tag="xt")
        for i in range(4):
            nc.sync.dma_start(out=wo_sb[:, 4 * i:4 * i + 4, :],
                              in_=wo[:, 4 * i:4 * i + 4, :])
        attn_round(2, 0, 4)
        recip_quarter(2)
        attn_round(3, 0, 3)
        # last quarter's reciprocals for pairs r=0..2 early (rows 0/32/64),
        # so only row 96 remains after the final pair
        hs3 = slice(3072, 4096)
        nc.scalar.activation(dstage[0:65, hs3], dstage[0:65, hs3], Ln)
        nc.scalar.activation(dstage[0:65, hs3], dstage[0:65, hs3], Exp,
                             scale=-1.0)
        for rr in range(3):
            nc.sync.dma_start(out=ddram[rr:rr + 1, hs3],
                              in_=dstage[32 * rr:32 * rr + 1, hs3])
        for fc in (12, 13, 14):
            df = (fc // 4) * 1024
            r2 = io.tile([128, TOK], bf16, tag="r2")
            nc.gpsimd.dma_start(
                out=r2[0:64, :],
                in_=ddram[fc % 4:fc % 4 + 1, df:df + 512].partition_broadcast(64))
            nc.gpsimd.dma_start(
                out=r2[64:128, :],
                in_=ddram[fc % 4:fc % 4 + 1, df + 512:df + 1024].partition_broadcast(64))
            nc.vector.tensor_mul(ot_sb[:, fc, :], ot_sb[:, fc, :], r2)
        attn_round(3, 3, 4)
        # warm-keepers: harmless matmuls bridge the final reciprocal chain so
        # the PE HAM clock stays at 8/8 and the output projection starts warm
        for _ in range(12):
            sdum = psum.tile([128, 1024], f32, tag="s")
            nc.tensor.matmul(
                sdum[0:65, 0:512], lhsT=vaug_sb[:, 0, 0:65],
                rhs=qt_sb[:, 0, :], start=True, stop=True,
            )
        nc.scalar.activation(dstage[96:97, hs3], dstage[96:97, hs3], Ln)
        nc.scalar.activation(dstage[96:97, hs3], dstage[96:97, hs3], Exp,
                             scale=-1.0)
        nc.sync.dma_start(out=ddram[3:4, hs3], in_=dstage[96:97, hs3])
        r2 = io.tile([128, TOK], bf16, tag="r2")
        nc.gpsimd.dma_start(
            out=r2[0:64, :],
            in_=ddram[3:4, 3072:3584].partition_broadcast(64))
        nc.gpsimd.dma_start(
            out=r2[64:128, :],
            in_=ddram[3:4, 3584:4096].partition_broadcast(64))
        nc.vector.tensor_mul(ot_sb[:, 15, :], ot_sb[:, 15, :], r2)

        # ---- output projection: out[tok, D] = o_norm @ wo.T
        for tb2 in range(4):
            tsl = slice(tb2 * 128, (tb2 + 1) * 128)
            for dc in range(4):
                dsl = slice(dc * 512, (dc + 1) * 512)
                ps = opsum.tile([128, 512], f32, tag="o")
                for fc in range(DC):
                    nc.tensor.matmul(
                        ps,
                        lhsT=ot_sb[:, fc, tsl],
                        rhs=wo_sb[:, fc, dsl],
                        start=(fc == 0), stop=(fc == DC - 1),
                    )
                osb = outw.tile([128, 512], f32, tag="osb")
                nc.vector.tensor_copy(osb, ps)
                nc.sync.dma_start(out=out[tsl, dsl], in_=osb)

    nc.compile()
    return nc


def _prep_shared(freqs_cos, freqs_sin, wqkv, wo):
    """Weight/table prep shared by all cores (token rotation applied later)."""
    cs = np.asarray(freqs_cos)[:, 0, :]  # [S, 64] (already repeat-2 layout)
    sn = np.asarray(freqs_sin)[:, 0, :]
    cos_h = np.empty((128, S), np.float32)
    sin_h = np.empty((128, S), np.float32)
    for p in range(128):
        cos_h[p] = cs[:, p % 64]
        sin_h[p] = sn[:, p % 64] * (-1.0 if p % 2 == 0 else 1.0)

    # Q rows permuted: fc = 4t+r -> [head 8t+r | head 8t+4+r]
    qrows = []
    for t in range(4):
        for r in range(4):
            for h in (8 * t + r, 8 * t + 4 + r):
                qrows.extend(range(h * HD, (h + 1) * HD))
    wq_t = np.ascontiguousarray(wqkv[qrows, :].T)  # [D, 2048]
    wq_h = np.ascontiguousarray(
        wq_t.reshape(DC, 128, DC, 128).transpose(1, 2, 0, 3)).astype(BF16)

    # K rows: tile t holds groups (2t | 2t+1)
    krows = []
    for t in range(4):
        for g in (2 * t, 2 * t + 1):
            krows.extend(range(H * HD + g * HD, H * HD + (g + 1) * HD))
    wk_t = np.ascontiguousarray(wqkv[krows, :].T)  # [D, 512]
    wk_h = np.ascontiguousarray(
        wk_t.reshape(DC, 128, 4, 128).transpose(1, 0, 2, 3)).astype(BF16)

    # V rows natural group order (cols t*128 : A 64 | B 64)
    vrows = list(range((H + G) * HD, (H + 2 * G) * HD))
    wv_t = np.ascontiguousarray(wqkv[vrows, :].T)  # [D, 512]
    wv_h = np.ascontiguousarray(
        wv_t.reshape(DC, 128, 512).transpose(1, 0, 2)).astype(BF16)

    # wo rhs: wo_h[p, fc, dcol] = wo[dcol, feat(fc, p)]
    feat = np.empty(D, np.int64)
    for fc in range(DC):
        t, r = divmod(fc, 4)
        for p in range(128):
            h = 8 * t + r + (4 if p >= 64 else 0)
            feat[fc * 128 + p] = h * HD + (p % 64)
    wo_h = np.ascontiguousarray(
        np.asarray(wo)[:, feat].T.reshape(DC, 128, D).transpose(1, 0, 2)
    ).astype(BF16)
    return cos_h, sin_h, wq_h, wk_h, wv_h, wo_h


def _prep_inputs(x, freqs_cos, freqs_sin, wqkv, wo):
    cos_h, sin_h, wq_h, wk_h, wv_h, wo_h = _prep_shared(
        freqs_cos, freqs_sin, wqkv, wo)
    x = np.asarray(x)
    ins = []
    for c in range(N_CORES):
        b, t4 = divmod(c, 4)
        q0 = t4 * TOK
        rot = (np.arange(S) + q0) % S  # own tokens land at cols 0:512
        xt_h = np.ascontiguousarray(
            x[b].T[:, rot].reshape(DC, 128, S).transpose(1, 0, 2)).astype(BF16)
        ins.append({
            "xt": xt_h,
            "wq": wq_h, "wk": wk_h, "wv": wv_h, "wo": wo_h,
            "cosr": np.ascontiguousarray(cos_h[:, rot]).astype(BF16),
            "sinr": np.ascontiguousarray(sin_h[:, rot]).astype(BF16),
        })
    return ins


TRACE = False


def kernel(x, freqs_cos, freqs_sin, wqkv, wo):
    if "nc" not in _CACHE:
        _CACHE["nc"] = _build()
    nc = _CACHE["nc"]
    ins = _prep_inputs(x, freqs_cos, freqs_sin, wqkv, wo)
    res = run_bass_kernel_spmd(nc, ins, list(range(N_CORES)), trace=TRACE)
    _CACHE["res"] = res
    out = np.empty((B, S, D), np.float32)
    for c in range(N_CORES):
        b, t4 = divmod(c, 4)
        out[b, t4 * TOK:(t4 + 1) * TOK, :] = res.results[c]["out"]
    return out


if __name__ == "__main__":
    rng = np.random.default_rng(0)
    x = rng.normal(size=(B, S, D)).astype(np.float32)
    fc_ = rng.random(size=(S, 1, HD)).astype(np.float32)
    fs_ = rng.random(size=(S, 1, HD)).astype(np.float32)
    wq_ = rng.normal(size=(3072, D)).astype(np.float32) * 0.02
    wo_ = rng.normal(size=(D, D)).astype(np.float32) * 0.02
    o = kernel(x, fc_, fs_, wq_, wo_)
    print(o.shape, o.dtype)

